# revision 4
# baseline (speedup 1.0000x reference)
"""Windowed sparse attention kernel for TRN2 (8 NeuronCores).

Problem: b=1, h=16, n=16384, d=32, window w=128, nw=128 windows.
Each window of 128 queries attends to [4 memory slots | prev window | cur window]
with additive bias, tanh softcap (50), softmax.

Sharding: sequence-parallel over windows. Core c handles windows
[c*16, (c+1)*16) for all 16 heads, with a one-window k/v halo.

Device dataflow (keys on partitions, packed slot-major, bf16 matmuls):
  sim columns (4096 per head): [s1(256) | s2 | ... | s15 | s16_cur(128) |
  s0_prev(128)], where slot s's 256-col block = [cur(q_{s-1}) | prev(q_s)].
  One matmul per slot computes simT[key_s, q-cols]. DVE adds the
  (pre-arranged, mask-folded) fp32 bias in 1024-col chunks while
  evacuating PSUM. ACT applies tanh softcap + exp in wide 4096-col
  instructions; exp output is bf16. mm2: lhsT = p-slice (keys x 128
  queries, bf16), rhs = v~ (keys x 33, bf16) -> out (128 q, 33) per task;
  v~'s ones column makes col 32 the softmax denominator Z. Host combines
  the 4-slot memory attention (1.5% of keys) and normalizes.
"""

import numpy as np
import ml_dtypes

BF16 = ml_dtypes.bfloat16

B, H, N, D = 1, 16, 16384, 32
W = 128                 # window size
NW = N // W             # 128 windows
NCORES = 8
WPC = NW // NCORES      # 16 windows (tasks) per core
NSLOT = WPC + 1         # 17 k/v slots (halo)
SOFTCLAMP = 50.0
SCALE = D ** -0.5
MASK_PEN = -30000.0
SIMW = WPC * 256        # 4096 packed sim cols per head

_COMPILED = None


def _prev_col(t):
    """Column of task t's prev-role 128-wide block in the packed layout."""
    return (t - 1) * 256 + 128 if t >= 1 else 15 * 256 + 128  # t=0 -> 3968


def _cur_col(t):
    """Column of task t's cur-role 128-wide block in the packed layout."""
    return t * 256 if t <= 14 else 15 * 256  # t=15 -> 3840


def _build_bass():
    import concourse.bacc as bacc
    import concourse.tile as tile
    from concourse import mybir
    from contextlib import ExitStack

    f32 = mybir.dt.float32
    bf16 = mybir.dt.bfloat16
    nc = bacc.Bacc()

    qT = nc.declare_dram_parameter("qT", [4, 128, WPC * W], bf16, isOutput=False)
    kT = nc.declare_dram_parameter("kT", [4, 128, NSLOT * W], bf16, isOutput=False)
    vv = nc.declare_dram_parameter("vv", [H, 128, NSLOT * 33], bf16, isOutput=False)
    bT = nc.declare_dram_parameter("bT", [128, SIMW], f32, isOutput=False)
    o = nc.declare_dram_parameter("o", [H, 128, WPC * 33], f32, isOutput=True)

    with ExitStack() as ctx:
        tc = ctx.enter_context(tile.TileContext(nc))
        singles = ctx.enter_context(tc.tile_pool(name="singles", bufs=1))
        qk_pool = ctx.enter_context(tc.tile_pool(name="qk", bufs=2))
        v_pool = ctx.enter_context(tc.tile_pool(name="v", bufs=2))
        wide = ctx.enter_context(tc.tile_pool(name="wide", bufs=3))
        ow_pool = ctx.enter_context(tc.tile_pool(name="ow", bufs=2))
        sim_ps = ctx.enter_context(tc.tile_pool(name="simps", bufs=3, space="PSUM"))
        out_ps = ctx.enter_context(tc.tile_pool(name="outps", bufs=2, space="PSUM"))

        # bias + v go on the scalar-engine HWDGE ring so the 2.1MB bias
        # transfer doesn't head-of-line-block the q/k loads (sync ring)
        # that the first matmuls need.
        biasS = singles.tile([128, SIMW], f32)

        for g in range(4):
            Qg = qk_pool.tile([128, WPC * W], bf16, tag="qg")
            nc.sync.dma_start(out=Qg[:, :], in_=qT[g])
            Kg = qk_pool.tile([128, NSLOT * W], bf16, tag="kg")
            nc.sync.dma_start(out=Kg[:, :], in_=kT[g])
            for i in range(4):
                h = 4 * g + i
                p0 = 32 * i
                Vh = v_pool.tile([128, NSLOT * 33], bf16)
                nc.scalar.dma_start(out=Vh[:, :], in_=vv[h])
                if h == 0:
                    nc.scalar.dma_start(out=biasS[:, :], in_=bT[:, :])

                simS = wide.tile([128, SIMW], f32, tag="simS")
                # mm1 into 4 PSUM chunks of 1024 cols (2 banks each).
                # chunk j covers packed cols [1024j, 1024j+1024).
                for j in range(4):
                    simP = sim_ps.tile([128, 1024], f32)
                    for s in range(4 * j + 1, 4 * j + 5):
                        # slot s block at packed col (s-1)*256, local offset:
                        off = (s - 1) * 256 - 1024 * j
                        lhsT = Kg[p0:p0 + 32, s * W:(s + 1) * W]
                        if s <= 15:
                            nc.tensor.matmul(simP[:, off:off + 256], lhsT=lhsT,
                                             rhs=Qg[p0:p0 + 32,
                                                    (s - 1) * W:(s + 1) * W],
                                             start=True, stop=True,
                                             tile_position=(p0, 0))
                        else:  # s == 16: cur-role only (task 15), 128 cols
                            nc.tensor.matmul(simP[:, off:off + 128], lhsT=lhsT,
                                             rhs=Qg[p0:p0 + 32, 15 * W:16 * W],
                                             start=True, stop=True,
                                             tile_position=(p0, 0))
                    if j == 3:
                        # slot 0 prev-role only (task 0) at packed col 3968
                        nc.tensor.matmul(simP[:, 896:1024],
                                         lhsT=Kg[p0:p0 + 32, 0:W],
                                         rhs=Qg[p0:p0 + 32, 0:W],
                                         start=True, stop=True,
                                         tile_position=(p0, 0))
                    nc.vector.tensor_add(
                        simS[:, j * 1024:(j + 1) * 1024],
                        simP[:, :],
                        biasS[:, j * 1024:(j + 1) * 1024],
                    )
                # softcap + exp, wide
                tS = wide.tile([128, SIMW], f32, tag="tS")
                nc.scalar.activation(tS[:, :], simS[:, :],
                                     mybir.ActivationFunctionType.Tanh,
                                     scale=1.0 / SOFTCLAMP)
                pS = wide.tile([128, SIMW], bf16, tag="pS")
                nc.scalar.activation(pS[:, :], tS[:, :],
                                     mybir.ActivationFunctionType.Exp,
                                     scale=SOFTCLAMP)
                # mm2: out (128 q, 33) per task, 8 tasks per PSUM bank
                outW = ow_pool.tile([128, WPC * 33], f32)
                for tb in range(2):
                    otP = out_ps.tile([128, 8 * 33], f32)
                    for u in range(8):
                        t = 8 * tb + u
                        pc = _prev_col(t)
                        cc = _cur_col(t)
                        nc.tensor.matmul(
                            otP[:, u * 33:(u + 1) * 33],
                            lhsT=pS[:, pc:pc + 128],
                            rhs=Vh[:, t * 33:(t + 1) * 33],
                            start=True, stop=False)
                        nc.tensor.matmul(
                            otP[:, u * 33:(u + 1) * 33],
                            lhsT=pS[:, cc:cc + 128],
                            rhs=Vh[:, (t + 1) * 33:(t + 2) * 33],
                            start=False, stop=True)
                    nc.vector.tensor_copy(outW[:, tb * 264:(tb + 1) * 264], otP[:, :])
                nc.sync.dma_start(out=o[h], in_=outW[:, :])
    nc.compile()
    return nc


def _get_compiled():
    global _COMPILED
    if _COMPILED is None:
        _COMPILED = _build_bass()
    return _COMPILED


def _prep_core(c, qs, ks, vs, ab, mvec):
    """Build per-core input arrays. qs,ks,vs: (H, N, D) (qs pre-scaled)."""
    w0 = c * WPC
    qw = qs.reshape(H, NW, W, D)[:, w0:w0 + WPC]          # (H,16,128,32)
    qTc = np.ascontiguousarray(
        qw.reshape(4, 4, WPC, W, D).transpose(0, 1, 4, 2, 3).reshape(4, 128, WPC * W))

    kw = ks.reshape(H, NW, W, D)
    vw = vs.reshape(H, NW, W, D)
    khalo = np.zeros((H, NSLOT, W, D), BF16)
    vhalo = np.zeros((H, NSLOT, W, D), BF16)
    lo = w0 - 1
    src_lo = max(lo, 0)
    dst_lo = src_lo - lo
    khalo[:, dst_lo:] = kw[:, src_lo:w0 + WPC]
    vhalo[:, dst_lo:] = vw[:, src_lo:w0 + WPC]
    kTc = np.ascontiguousarray(
        khalo.reshape(4, 4, NSLOT, W, D).transpose(0, 1, 4, 2, 3).reshape(4, 128, NSLOT * W))
    vvc = np.concatenate([vhalo, np.ones((H, NSLOT, W, 1), BF16)], axis=3)
    vvc = np.ascontiguousarray(
        vvc.transpose(0, 2, 1, 3).reshape(H, 128, NSLOT * 33))

    # bias, packed layout: slot s (1..15) block at col (s-1)*256 =
    # [cur-bias(task s-1) | prev-bias(task s)]; slot 16 cur at 3840;
    # slot 0 prev at 3968. Key mask (+ structural masking of window -1)
    # folded as additive penalty; keys of block s = global window w0+s-1.
    bTc = np.zeros((128, SIMW), np.float32)                # (key, col)
    def pen(gw):
        if gw < 0:
            return np.full((W,), MASK_PEN, np.float32)
        return np.where(mvec[gw * W:(gw + 1) * W], np.float32(0),
                        np.float32(MASK_PEN))
    for s in range(1, 16):
        gw = w0 + s - 1
        base = (s - 1) * 256
        bTc[:, base:base + 128] = ab[gw, :, 128:256].T      # cur role, task s-1
        bTc[:, base + 128:base + 256] = ab[gw + 1, :, 0:128].T  # prev role, task s
        bTc[:, base:base + 256] += pen(gw)[:, None]
    bTc[:, 3840:3968] = ab[w0 + 15, :, 128:256].T + pen(w0 + 15)[:, None]
    bTc[:, 3968:4096] = ab[w0, :, 0:128].T + pen(w0 - 1)[:, None]
    return {"qT": qTc, "kT": kTc, "vv": vvc, "bT": bTc}


def _run_device(in_maps, trace=False):
    from concourse.bass_utils import run_bass_kernel_spmd
    nc = _get_compiled()
    res = run_bass_kernel_spmd(nc, in_maps, list(range(NCORES)), trace=trace)
    return res


def kernel(q, k, v, mask, attn_bias, memory_kv, _trace=False, _ret_res=False):
    q = np.asarray(q, np.float32)
    k = np.asarray(k, np.float32)
    v = np.asarray(v, np.float32)
    mask = np.asarray(mask)
    attn_bias = np.asarray(attn_bias, np.float32)
    memory_kv = np.asarray(memory_kv, np.float32)

    qs = (q[0] * np.float32(SCALE)).astype(BF16)   # (H, N, D)
    ks, vs = k[0].astype(BF16), v[0].astype(BF16)
    ab = attn_bias[0]                   # (NW, W, 2W)
    mvec = mask[0].astype(bool)         # (N,)

    in_maps = [_prep_core(c, qs, ks, vs, ab, mvec) for c in range(NCORES)]
    res = _run_device(in_maps, trace=_trace)
    outs = [r["o"] for r in res.results]             # each (H, 128, WPC*33)

    big = np.stack(outs)                              # (8, H, 128, 528)
    # (core, h, q, task, 33) -> (h, core, task, q, 33) -> (h, n, 33)
    arr = big.reshape(NCORES, H, W, WPC, 33).transpose(1, 0, 3, 2, 4)
    arr = arr.reshape(H, N, 33)
    num = arr[..., :D].astype(np.float64)             # (H, N, D)
    z = arr[..., D].astype(np.float64)                # (H, N)

    # memory-slot attention (4 keys, no bias, mask=True) on host
    mk, mv = memory_kv[0], memory_kv[1]               # (H, 4, D)
    qs64 = q[0].astype(np.float64) * SCALE
    sim_m = np.einsum('hnd,hmd->hnm', qs64, mk.astype(np.float64))
    pm = np.exp(SOFTCLAMP * np.tanh(sim_m / SOFTCLAMP))
    num = num + np.einsum('hnm,hmd->hnd', pm, mv.astype(np.float64))
    z = z + pm.sum(-1)

    out = (num / z[..., None]).astype(np.float32)[None]   # (1, H, N, D)
    if _ret_res:
        return out, res
    return out


# revision 7
# speedup vs baseline: 1.2259x; 1.2259x over previous
"""Windowed sparse attention kernel for TRN2 (8 NeuronCores).

Problem: b=1, h=16, n=16384, d=32, window w=128, nw=128 windows.
Each window of 128 queries attends to [4 memory slots | prev window | cur window]
with additive bias, tanh softcap (50), softmax.

Sharding: sequence-parallel over windows. Core c handles windows
[c*16, (c+1)*16) for all 16 heads, with a one-window k/v halo.

Device dataflow (keys on partitions, packed slot-major, bf16 matmuls):
  sim columns (4096 per head): [s1(256) | s2 | ... | s15 | s16_cur(128) |
  s0_prev(128)], where slot s's 256-col block = [cur(q_{s-1}) | prev(q_s)].
  One matmul per slot computes simT[key_s, q-cols]. DVE adds the
  (pre-arranged, mask-folded) fp32 bias in 1024-col chunks while
  evacuating PSUM. ACT applies tanh softcap + exp in wide 4096-col
  instructions; exp output is bf16. mm2: lhsT = p-slice (keys x 128
  queries, bf16), rhs = v~ (keys x 33, bf16) -> out (128 q, 33) per task;
  v~'s ones column makes col 32 the softmax denominator Z. Host combines
  the 4-slot memory attention (1.5% of keys) and normalizes.
"""

import numpy as np
import ml_dtypes

BF16 = ml_dtypes.bfloat16

B, H, N, D = 1, 16, 16384, 32
W = 128                 # window size
NW = N // W             # 128 windows
NCORES = 8
WPC = NW // NCORES      # 16 windows (tasks) per core
NSLOT = WPC + 1         # 17 k/v slots (halo)
SOFTCLAMP = 50.0
SCALE = D ** -0.5
MASK_PEN = -30000.0
SIMW = WPC * 256        # 4096 packed sim cols per head

_COMPILED = None


def _prev_col(t):
    """Column of task t's prev-role 128-wide block in the packed layout."""
    return (t - 1) * 256 + 128 if t >= 1 else 15 * 256 + 128  # t=0 -> 3968


def _cur_col(t):
    """Column of task t's cur-role 128-wide block in the packed layout."""
    return t * 256 if t <= 14 else 15 * 256  # t=15 -> 3840


def _build_bass():
    import concourse.bacc as bacc
    import concourse.tile as tile
    from concourse import mybir
    from contextlib import ExitStack

    f32 = mybir.dt.float32
    bf16 = mybir.dt.bfloat16
    nc = bacc.Bacc()

    qT = nc.declare_dram_parameter("qT", [4, 128, WPC * W], bf16, isOutput=False)
    kT = nc.declare_dram_parameter("kT", [4, 128, NSLOT * W], bf16, isOutput=False)
    vv = nc.declare_dram_parameter("vv", [H, 128, NSLOT * 33], bf16, isOutput=False)
    bT = nc.declare_dram_parameter("bT", [128, SIMW], f32, isOutput=False)
    o = nc.declare_dram_parameter("o", [H, 128, WPC * 33], f32, isOutput=True)

    with ExitStack() as ctx:
        tc = ctx.enter_context(tile.TileContext(nc))
        singles = ctx.enter_context(tc.tile_pool(name="singles", bufs=1))
        qk_pool = ctx.enter_context(tc.tile_pool(name="qk", bufs=2))
        v_pool = ctx.enter_context(tc.tile_pool(name="v", bufs=2))
        wide = ctx.enter_context(tc.tile_pool(name="wide", bufs=3))
        ow_pool = ctx.enter_context(tc.tile_pool(name="ow", bufs=2))
        sim_ps = ctx.enter_context(tc.tile_pool(name="simps", bufs=3, space="PSUM"))
        out_ps = ctx.enter_context(tc.tile_pool(name="outps", bufs=2, space="PSUM"))

        # bias in 4 independently-tracked chunk tiles, DMA'd after group-0
        # q/k so the first matmuls aren't head-of-line-blocked and each
        # DVE bias-add only waits for its own chunk.
        bias0 = singles.tile([128, 1024], f32, tag="bias0")
        bias1 = singles.tile([128, 1024], f32, tag="bias1")
        bias2 = singles.tile([128, 1024], f32, tag="bias2")
        bias3 = singles.tile([128, 1024], f32, tag="bias3")
        biasC = [bias0, bias1, bias2, bias3]

        for g in range(4):
            Qg = qk_pool.tile([128, WPC * W], bf16, tag="qg")
            nc.sync.dma_start(out=Qg[:, :], in_=qT[g])
            Kg = qk_pool.tile([128, NSLOT * W], bf16, tag="kg")
            nc.sync.dma_start(out=Kg[:, :], in_=kT[g])
            if g == 0:
                for j in range(4):
                    nc.sync.dma_start(out=biasC[j][:, :],
                                      in_=bT[:, 1024 * j:1024 * (j + 1)])
            for i in range(4):
                h = 4 * g + i
                p0 = 32 * i
                Vh = v_pool.tile([128, NSLOT * 33], bf16)
                nc.sync.dma_start(out=Vh[:, :], in_=vv[h])

                simS = wide.tile([128, SIMW], f32, tag="simS")
                # mm1 into 4 PSUM chunks of 1024 cols (2 banks each).
                # chunk j covers packed cols [1024j, 1024j+1024).
                for j in range(4):
                    simP = sim_ps.tile([128, 1024], f32)
                    for s in range(4 * j + 1, 4 * j + 5):
                        # slot s block at packed col (s-1)*256, local offset:
                        off = (s - 1) * 256 - 1024 * j
                        lhsT = Kg[p0:p0 + 32, s * W:(s + 1) * W]
                        if s <= 15:
                            nc.tensor.matmul(simP[:, off:off + 256], lhsT=lhsT,
                                             rhs=Qg[p0:p0 + 32,
                                                    (s - 1) * W:(s + 1) * W],
                                             start=True, stop=True,
                                             tile_position=(p0, 0))
                        else:  # s == 16: cur-role only (task 15), 128 cols
                            nc.tensor.matmul(simP[:, off:off + 128], lhsT=lhsT,
                                             rhs=Qg[p0:p0 + 32, 15 * W:16 * W],
                                             start=True, stop=True,
                                             tile_position=(p0, 0))
                    if j == 3:
                        # slot 0 prev-role only (task 0) at packed col 3968
                        nc.tensor.matmul(simP[:, 896:1024],
                                         lhsT=Kg[p0:p0 + 32, 0:W],
                                         rhs=Qg[p0:p0 + 32, 0:W],
                                         start=True, stop=True,
                                         tile_position=(p0, 0))
                    nc.vector.tensor_add(
                        simS[:, j * 1024:(j + 1) * 1024],
                        simP[:, :],
                        biasC[j][:, :],
                    )
                # softcap + exp, wide
                tS = wide.tile([128, SIMW], f32, tag="tS")
                nc.scalar.activation(tS[:, :], simS[:, :],
                                     mybir.ActivationFunctionType.Tanh,
                                     scale=1.0 / SOFTCLAMP)
                pS = wide.tile([128, SIMW], bf16, tag="pS")
                nc.scalar.activation(pS[:, :], tS[:, :],
                                     mybir.ActivationFunctionType.Exp,
                                     scale=SOFTCLAMP)
                # mm2: out (128 q, 33) per task, 8 tasks per PSUM bank
                outW = ow_pool.tile([128, WPC * 33], f32)
                for tb in range(2):
                    otP = out_ps.tile([128, 8 * 33], f32)
                    for u in range(8):
                        t = 8 * tb + u
                        pc = _prev_col(t)
                        cc = _cur_col(t)
                        nc.tensor.matmul(
                            otP[:, u * 33:(u + 1) * 33],
                            lhsT=pS[:, pc:pc + 128],
                            rhs=Vh[:, t * 33:(t + 1) * 33],
                            start=True, stop=False)
                        nc.tensor.matmul(
                            otP[:, u * 33:(u + 1) * 33],
                            lhsT=pS[:, cc:cc + 128],
                            rhs=Vh[:, (t + 1) * 33:(t + 2) * 33],
                            start=False, stop=True)
                    nc.vector.tensor_copy(outW[:, tb * 264:(tb + 1) * 264], otP[:, :])
                nc.sync.dma_start(out=o[h], in_=outW[:, :])
    nc.compile()
    return nc


def _get_compiled():
    global _COMPILED
    if _COMPILED is None:
        _COMPILED = _build_bass()
    return _COMPILED


def _prep_core(c, qs, ks, vs, ab, mvec):
    """Build per-core input arrays. qs,ks,vs: (H, N, D) (qs pre-scaled)."""
    w0 = c * WPC
    qw = qs.reshape(H, NW, W, D)[:, w0:w0 + WPC]          # (H,16,128,32)
    qTc = np.ascontiguousarray(
        qw.reshape(4, 4, WPC, W, D).transpose(0, 1, 4, 2, 3).reshape(4, 128, WPC * W))

    kw = ks.reshape(H, NW, W, D)
    vw = vs.reshape(H, NW, W, D)
    khalo = np.zeros((H, NSLOT, W, D), BF16)
    vhalo = np.zeros((H, NSLOT, W, D), BF16)
    lo = w0 - 1
    src_lo = max(lo, 0)
    dst_lo = src_lo - lo
    khalo[:, dst_lo:] = kw[:, src_lo:w0 + WPC]
    vhalo[:, dst_lo:] = vw[:, src_lo:w0 + WPC]
    kTc = np.ascontiguousarray(
        khalo.reshape(4, 4, NSLOT, W, D).transpose(0, 1, 4, 2, 3).reshape(4, 128, NSLOT * W))
    vvc = np.concatenate([vhalo, np.ones((H, NSLOT, W, 1), BF16)], axis=3)
    vvc = np.ascontiguousarray(
        vvc.transpose(0, 2, 1, 3).reshape(H, 128, NSLOT * 33))

    # bias, packed layout: slot s (1..15) block at col (s-1)*256 =
    # [cur-bias(task s-1) | prev-bias(task s)]; slot 16 cur at 3840;
    # slot 0 prev at 3968. Key mask (+ structural masking of window -1)
    # folded as additive penalty; keys of block s = global window w0+s-1.
    bTc = np.zeros((128, SIMW), np.float32)                # (key, col)
    def pen(gw):
        if gw < 0:
            return np.full((W,), MASK_PEN, np.float32)
        return np.where(mvec[gw * W:(gw + 1) * W], np.float32(0),
                        np.float32(MASK_PEN))
    for s in range(1, 16):
        gw = w0 + s - 1
        base = (s - 1) * 256
        bTc[:, base:base + 128] = ab[gw, :, 128:256].T      # cur role, task s-1
        bTc[:, base + 128:base + 256] = ab[gw + 1, :, 0:128].T  # prev role, task s
        bTc[:, base:base + 256] += pen(gw)[:, None]
    bTc[:, 3840:3968] = ab[w0 + 15, :, 128:256].T + pen(w0 + 15)[:, None]
    bTc[:, 3968:4096] = ab[w0, :, 0:128].T + pen(w0 - 1)[:, None]
    return {"qT": qTc, "kT": kTc, "vv": vvc, "bT": bTc}


def _run_device(in_maps, trace=False):
    from concourse.bass_utils import run_bass_kernel_spmd
    nc = _get_compiled()
    res = run_bass_kernel_spmd(nc, in_maps, list(range(NCORES)), trace=trace)
    return res


def kernel(q, k, v, mask, attn_bias, memory_kv, _trace=False, _ret_res=False):
    q = np.asarray(q, np.float32)
    k = np.asarray(k, np.float32)
    v = np.asarray(v, np.float32)
    mask = np.asarray(mask)
    attn_bias = np.asarray(attn_bias, np.float32)
    memory_kv = np.asarray(memory_kv, np.float32)

    qs = (q[0] * np.float32(SCALE)).astype(BF16)   # (H, N, D)
    ks, vs = k[0].astype(BF16), v[0].astype(BF16)
    ab = attn_bias[0]                   # (NW, W, 2W)
    mvec = mask[0].astype(bool)         # (N,)

    in_maps = [_prep_core(c, qs, ks, vs, ab, mvec) for c in range(NCORES)]
    res = _run_device(in_maps, trace=_trace)
    outs = [r["o"] for r in res.results]             # each (H, 128, WPC*33)

    big = np.stack(outs)                              # (8, H, 128, 528)
    # (core, h, q, task, 33) -> (h, core, task, q, 33) -> (h, n, 33)
    arr = big.reshape(NCORES, H, W, WPC, 33).transpose(1, 0, 3, 2, 4)
    arr = arr.reshape(H, N, 33)
    num = arr[..., :D].astype(np.float64)             # (H, N, D)
    z = arr[..., D].astype(np.float64)                # (H, N)

    # memory-slot attention (4 keys, no bias, mask=True) on host
    mk, mv = memory_kv[0], memory_kv[1]               # (H, 4, D)
    qs64 = q[0].astype(np.float64) * SCALE
    sim_m = np.einsum('hnd,hmd->hnm', qs64, mk.astype(np.float64))
    pm = np.exp(SOFTCLAMP * np.tanh(sim_m / SOFTCLAMP))
    num = num + np.einsum('hnm,hmd->hnd', pm, mv.astype(np.float64))
    z = z + pm.sum(-1)

    out = (num / z[..., None]).astype(np.float32)[None]   # (1, H, N, D)
    if _ret_res:
        return out, res
    return out
